# revision 1
# baseline (speedup 1.0000x reference)
"""CenterNet (CtdetLoss) Trainium2 Bass kernel.

Math: with p = pred_hm, t = log1p(-p) * p^2, m4 = (1-hm)^4,
  F - Z = t*(m4-1)  densely, plus  ln(p)*(1-p)^2  at the K-sparse
  positive pixels (hm == 1.0, which are exactly the object centers).
Per-object rectangle sums are computed without summed-area tables:
  rect_k(channel c_k) = sum_y My[k,y] * sum_x Mx[k,x] * G[c_k,y,x]
The y-contraction runs on the TensorEngine (lhsT = My^T 0/1 matrix),
the x-mask on the VectorEngine, and the class selection is a one-hot
mask reduce over the accumulated [K, C] table. The class-summed Z map
for S_ZS is PE-accumulated into a column-folded [K, 512] PSUM tile.

Engine assignment for the dense pipeline (per [H, 16*W] tile):
  ACT : l1 = ln(1-p), m2 = (1-hm)^2, m4 = m2^2     (3 passes, bf16 out)
  Pool: p2 = p*p                                    (gpsimd)
  DVE : t = l1*p2,  r = t*m4                        (bf16 tt, 2x mode)
  PE  : psz += My^T @ t;  psg = My^T @ r + (-My)^T @ t  (g4 = r - t is
        never materialized; the -t term uses a negated-My lhsT)
  DVE : masked = psg * Mx2 (broadcast), V[:,c] = reduce_x(masked)
The per-class matmuls fold x-pairs (even/odd rhs strides accumulate
into the same PSUM), halving the x-mask reduce; Mx2 is the pair-mean
of the 0/1 Mx mask (~1e-4 relative error at rect boundaries).

The heatmaps are pre-transposed on host to [NB, H, C, W] (y-major) so
every dense DMA moves 16*W*4 = 8KB contiguous runs per partition.
Positive-pixel values and the reg-L1 values are fetched with indirect
row-gather DMAs and column-selected with shipped one-hot masks. The
two images' tile loops are interleaved so each engine always has an
independent tile in flight.

Sharding: data-parallel over batch, 2 images per core on 8 cores. Host
preprocessing only touches the small int tensors (masks, one-hots,
gather row indices, per-object weights) plus a pure layout transpose
of the dense maps; every FLOP on dense map data runs on device. Host
combines the 8 cores' per-image partial sums into the final 4 scalars.
"""

import sys

sys.path.insert(0, "/opt/trn_rl_repo")

import numpy as np
import ml_dtypes

B, C, H, W, K = 16, 80, 128, 128, 128
NCORES = 8
NB = B // NCORES          # images per core
CG = 16                   # channels per DMA tile
NG = C // CG              # dense tiles per image
HM_W, WH_W, OFF_W = 1.0, 0.1, 1.0

BF16 = ml_dtypes.bfloat16

# packed f32 const columns: mxr4 | mx2 | cxsel | csind | m2 | tmw | tmr | sk
F_MXR, F_MX2, F_CX = 0, 4 * W, 4 * W + W // 2
F_CS = F_CX + W
F_M2 = F_CS + W
F_TW, F_TR, F_SK = F_M2 + 2, F_M2 + 4, F_M2 + 6
F_TOT = F_M2 + 7
# packed bf16 const columns: myt | myn (= -myt) | eoh | mts
B_MY, B_MN, B_EO = 0, K, 2 * K
B_MT = 2 * K + C
B_TOT = 2 * K + C + K

_module_cache = {}


def build_module():
    """Build (once) the per-core Bass module. Returns nc."""
    if "nc" in _module_cache:
        return _module_cache["nc"]

    import concourse.bacc as bacc
    import concourse.bass as bass
    import concourse.tile as tile
    from concourse import mybir

    f32 = mybir.dt.float32
    bf16 = mybir.dt.bfloat16
    i32 = mybir.dt.int32
    Alu = mybir.AluOpType
    Act = mybir.ActivationFunctionType
    Ax = mybir.AxisListType

    nc = bacc.Bacc(None, target_bir_lowering=False)

    # ---- DRAM I/O ----
    phm = nc.dram_tensor("phm", [NB, H, C, W], f32, kind="ExternalInput")
    hmt = nc.dram_tensor("hm", [NB, H, C, W], f32, kind="ExternalInput")
    pwh = nc.dram_tensor("pwh", [NB, H, 2, W], f32, kind="ExternalInput")
    prg = nc.dram_tensor("prg", [NB, H, 2, W], f32, kind="ExternalInput")
    fpk = nc.dram_tensor("fpk", [NB, K, F_TOT], f32, kind="ExternalInput")
    bpk = nc.dram_tensor("bpk", [NB, K, B_TOT], bf16, kind="ExternalInput")
    ipk = nc.dram_tensor("ipk", [NB, K, 2], i32, kind="ExternalInput")
    out = nc.dram_tensor("out", [4, NB], f32, kind="ExternalOutput")

    phm_rows = phm[:].rearrange("b y c x -> (b y c) x")
    pwh_rows = pwh[:].rearrange("b y d x -> (b y) (d x)")
    prg_rows = prg[:].rearrange("b y d x -> (b y) (d x)")

    with tile.TileContext(nc) as tc:
        with (
            tc.tile_pool(name="consts", bufs=1) as consts,
            tc.tile_pool(name="work", bufs=3) as work,
            tc.tile_pool(name="scr", bufs=4) as scr,
            tc.tile_pool(name="acc", bufs=1) as acc,
            tc.tile_pool(name="ep", bufs=2) as ep,
            tc.tile_pool(name="psb", bufs=3, space="PSUM") as psb,
            tc.tile_pool(name="psz", bufs=1, space="PSUM") as pszp,
            tc.tile_pool(name="pss", bufs=1, space="PSUM") as pss,
        ):
            ones_s = consts.tile([K, 1], f32, tag="ones")
            nc.vector.memset(ones_s, 1.0)
            O = acc.tile([4, NB], f32, tag="O")

            # ---- per-image constants (3 packed DMAs each) ----
            hd = []
            for b in range(NB):
                fp_s = consts.tile([K, F_TOT], f32, tag=f"fp{b}")
                nc.sync.dma_start(out=fp_s, in_=fpk[b])
                bp_s = consts.tile([K, B_TOT], bf16, tag=f"bp{b}")
                nc.sync.dma_start(out=bp_s, in_=bpk[b])
                ip_s = consts.tile([K, 2], i32, tag=f"ip{b}")
                nc.sync.dma_start(out=ip_s, in_=ipk[b])
                hd.append(
                    dict(
                        myt=bp_s[:, B_MY : B_MY + K],
                        myn=bp_s[:, B_MN : B_MN + K],
                        eoh=bp_s[:, B_EO : B_EO + C],
                        mts=bp_s[:, B_MT : B_MT + K],
                        mxr4=fp_s[:, F_MXR : F_MXR + 4 * W],
                        mx2=fp_s[:, F_MX2 : F_MX2 + W // 2],
                        cxsel=fp_s[:, F_CX : F_CX + W],
                        csind=fp_s[:, F_CS : F_CS + W],
                        m2c=fp_s[:, F_M2 : F_M2 + 2],
                        tmw=fp_s[:, F_TW : F_TW + 2],
                        tmr=fp_s[:, F_TR : F_TR + 2],
                        sk=fp_s[:, F_SK : F_SK + 1],
                        rpos=ip_s[:, 0:1],
                        rind=ip_s[:, 1:2],
                        V=acc.tile([K, C], bf16, tag=f"V{b}", name=f"V{b}"),
                        psz=pszp.tile([K, 4 * W], f32, tag=f"pszacc{b}", name=f"psz{b}"),
                    )
                )

            # ---- dense tile loop (16 channels per tile), images interleaved
            for g in range(NG):
                cs = g * CG
                for b in range(NB):
                    h = hd[b]
                    myt, myn, mx2, V, psz_acc = (
                        h["myt"], h["myn"], h["mx2"], h["V"], h["psz"],
                    )
                    p16 = work.tile([H, CG * W], f32, tag="p16")
                    nc.sync.dma_start(
                        out=p16[:].rearrange("p (c x) -> p c x", c=CG),
                        in_=phm[b, :, cs : cs + CG],
                    )
                    h16 = work.tile([H, CG * W], f32, tag="h16")
                    nc.sync.dma_start(
                        out=h16[:].rearrange("p (c x) -> p c x", c=CG),
                        in_=hmt[b, :, cs : cs + CG],
                    )
                    # t = log1p(-p) * p^2 ; r = t * (1-hm)^4
                    l1 = work.tile([H, CG * W], bf16, tag="l1")
                    nc.scalar.activation(l1, p16, Act.Ln, bias=1.0, scale=-1.0)
                    m2t = work.tile([H, CG * W], bf16, tag="m2t")
                    nc.scalar.activation(m2t, h16, Act.Square, bias=1.0, scale=-1.0)
                    m4t = work.tile([H, CG * W], bf16, tag="m4t")
                    nc.scalar.activation(m4t, m2t, Act.Square)
                    p2 = work.tile([H, CG * W], bf16, tag="p2")
                    nc.gpsimd.tensor_mul(p2, p16, p16)
                    t = work.tile([H, CG * W], bf16, tag="t")
                    nc.vector.tensor_mul(t, l1, p2)
                    r = work.tile([H, CG * W], bf16, tag="r")
                    nc.vector.tensor_mul(r, t, m4t)
                    # S_ZS accumulation on PE: psz_acc += MyT.T @ t chunks
                    # (all chunks fold onto the same 512 columns; the final
                    # Mx mask reduce sums out the channel residues)
                    for ch in range(4):
                        nc.tensor.matmul(
                            psz_acc,
                            lhsT=myt,
                            rhs=t[:, ch * 512 : ch * 512 + 512],
                            start=(g == 0 and ch == 0),
                            stop=(g == NG - 1 and ch == 3),
                            skip_group_check=True,
                        )
                    # per-class rects with x-pair folding: the even/odd
                    # matmuls accumulate into the same PSUM so the x-mask
                    # reduce runs on half-width data. g4 = r - t is never
                    # materialized: the -t term uses the negated-My lhsT.
                    for hf in range(2):
                        c0 = cs + hf * 8
                        psgh = psb.tile([K, 8 * (W // 2)], f32, tag="psgh")
                        rv = r[:, hf * 1024 : hf * 1024 + 1024].rearrange(
                            "p (c x t) -> p c x t", c=8, t=2
                        )
                        tv = t[:, hf * 1024 : hf * 1024 + 1024].rearrange(
                            "p (c x t) -> p c x t", c=8, t=2
                        )
                        nc.tensor.matmul(
                            psgh, lhsT=myt, rhs=rv[:, :, :, 0],
                            start=True, stop=False, skip_group_check=True,
                        )
                        nc.tensor.matmul(
                            psgh, lhsT=myt, rhs=rv[:, :, :, 1],
                            start=False, stop=False, skip_group_check=True,
                        )
                        nc.tensor.matmul(
                            psgh, lhsT=myn, rhs=tv[:, :, :, 0],
                            start=False, stop=False, skip_group_check=True,
                        )
                        nc.tensor.matmul(
                            psgh, lhsT=myn, rhs=tv[:, :, :, 1],
                            start=False, stop=True, skip_group_check=True,
                        )
                        masked = scr.tile([K, 8 * (W // 2)], bf16, tag="masked")
                        nc.vector.scalar_tensor_tensor(
                            masked[:].rearrange("k (c x) -> k c x", c=8),
                            psgh[:].rearrange("k (c x) -> k c x", c=8),
                            1.0,
                            mx2.unsqueeze(1).broadcast_to([K, 8, W // 2]),
                            op0=Alu.mult,
                            op1=Alu.mult,
                        )
                        with nc.allow_low_precision("bf16 V rect table"):
                            nc.vector.tensor_reduce(
                                V[:, c0 : c0 + 8],
                                masked[:].rearrange("k (c x) -> k c x", c=8),
                                axis=Ax.X,
                                op=Alu.add,
                            )

            # ---- per-image epilogues ----
            for b in range(NB):
                h = hd[b]
                V, psz_acc = h["V"], h["psz"]
                # S_ZS: Mx-masked reduce of the PE-accumulated psz_acc
                szs = ep.tile([K, 1], f32, tag=f"szs{b}")
                sc512 = scr.tile([K, 4 * W], f32, tag="scr512")
                nc.vector.scalar_tensor_tensor(
                    sc512,
                    psz_acc,
                    1.0,
                    h["mxr4"],
                    op0=Alu.mult,
                    op1=Alu.mult,
                    accum_out=szs,
                )
                # class-select rect sums: rectG[k] = sum_c V[k,c]*onehot[k,c]
                rectG = ep.tile([K, 1], f32, tag=f"rectG{b}")
                sc80 = scr.tile([K, C], bf16, tag="scr80")
                nc.vector.scalar_tensor_tensor(
                    sc80, V, 1.0, h["eoh"], op0=Alu.mult, op1=Alu.mult,
                    accum_out=rectG,
                )
                # positive-pixel term: gather pred_hm rows at the unique
                # centers, column-select, A = ln(p)*(1-p)^2, posG = MT.T @ A
                rowg = ep.tile([K, W], f32, tag=f"rowg{b}")
                nc.gpsimd.indirect_dma_start(
                    out=rowg,
                    out_offset=None,
                    in_=phm_rows,
                    in_offset=bass.IndirectOffsetOnAxis(ap=h["rpos"], axis=0),
                )
                pj = ep.tile([K, 1], f32, tag=f"pj{b}")
                sc = scr.tile([K, W], f32, tag="scr128")
                nc.vector.scalar_tensor_tensor(
                    sc, rowg, 1.0, h["cxsel"], op0=Alu.mult, op1=Alu.mult,
                    accum_out=pj,
                )
                lnp = ep.tile([K, 1], f32, tag=f"lnp{b}")
                nc.scalar.activation(lnp, pj, Act.Ln)
                q2 = ep.tile([K, 1], f32, tag=f"q2{b}")
                nc.scalar.activation(q2, pj, Act.Square, bias=1.0, scale=-1.0)
                A = ep.tile([K, 1], bf16, tag=f"A{b}")
                nc.vector.tensor_mul(A, lnp, q2)
                psp = pss.tile([K, 1], f32, tag="psp")
                nc.tensor.matmul(psp, lhsT=h["mts"], rhs=A, start=True, stop=True)
                # total = rectG + posG + S_ZS ;  Q[:,0] = total * s
                tot = ep.tile([K, 1], f32, tag=f"tot{b}")
                nc.vector.tensor_add(tot, rectG, psp)
                nc.vector.tensor_add(tot, tot, szs)
                Q = ep.tile([K, 4], f32, tag=f"Q{b}")
                nc.vector.memset(Q, 0.0)
                nc.vector.tensor_mul(Q[:, 0:1], tot, h["sk"])
                # reg-L1 columns: one [K, 2W] row gather per map (d folded)
                for col, rows_ap, tm in (
                    (1, pwh_rows, h["tmw"]),
                    (2, prg_rows, h["tmr"]),
                ):
                    rg = ep.tile([K, 2 * W], f32, tag=f"rg{col}{b}")
                    nc.gpsimd.indirect_dma_start(
                        out=rg,
                        out_offset=None,
                        in_=rows_ap,
                        in_offset=bass.IndirectOffsetOnAxis(ap=h["rind"], axis=0),
                    )
                    PW = ep.tile([K, 2], f32, tag=f"PW{col}{b}")
                    for d in range(2):
                        sc = scr.tile([K, W], f32, tag="scr128")
                        nc.vector.scalar_tensor_tensor(
                            sc,
                            rg[:, d * W : d * W + W],
                            1.0,
                            h["csind"],
                            op0=Alu.mult,
                            op1=Alu.mult,
                            accum_out=PW[:, d : d + 1],
                        )
                    u = ep.tile([K, 2], f32, tag=f"u{col}{b}")
                    nc.vector.tensor_mul(u, PW, h["m2c"])
                    nc.vector.tensor_sub(u, u, tm)
                    nc.vector.tensor_reduce(
                        Q[:, col : col + 1],
                        u,
                        axis=Ax.X,
                        op=Alu.add,
                        apply_absolute_value=True,
                    )
                # partition-reduce the 4 columns: out[4,1] = Q.T @ ones
                psq = pss.tile([4, 1], f32, tag="psq")
                nc.tensor.matmul(psq, lhsT=Q, rhs=ones_s, start=True, stop=True)
                nc.scalar.copy(O[:, b : b + 1], psq)

            nc.sync.dma_start(out=out[:], in_=O)

    nc.compile()
    _module_cache["nc"] = nc
    return nc


def prep_in_maps(inputs):
    """Host-side prep: shard + transpose the dense maps per core, derive
    mask/index constants from the small int tensors."""
    pred_hm = np.asarray(inputs["pred_hm"], np.float32)
    pred_wh = np.asarray(inputs["pred_wh"], np.float32)
    pred_reg = np.asarray(inputs["pred_reg"], np.float32)
    hm = np.asarray(inputs["hm"], np.float32)
    wh_t = np.asarray(inputs["wh_t"], np.float32)
    reg_t = np.asarray(inputs["reg_t"], np.float32)
    reg_mask = np.asarray(inputs["reg_mask"], np.float32)
    ind = np.asarray(inputs["ind"]).astype(np.int64)
    cxcy = np.asarray(inputs["cxcy"]).astype(np.int64)
    ori_wh = np.asarray(inputs["ori_wh"]).astype(np.int64)
    cls_idx = np.asarray(inputs["cls_idx"]).astype(np.int64)

    yy = np.arange(H)
    xx = np.arange(W)
    per_img = []
    for b in range(B):
        cls = cls_idx[b]
        cx, cy = cxcy[b, :, 0], cxcy[b, :, 1]
        w = wh_t[b, :, 0].astype(np.int64)
        h = wh_t[b, :, 1].astype(np.int64)
        y0 = np.maximum(1, cy - h // 2 - 1)
        y1 = np.minimum(H - 1, cy + h // 2 + 1)
        y1 = np.maximum(y1, y0)
        x0 = np.maximum(1, cx - w // 2 - 1)
        x1 = np.minimum(W - 1, cx + w // 2 + 1)
        x1 = np.maximum(x1, x0)

        MyT = ((yy[:, None] >= y0[None, :]) & (yy[:, None] < y1[None, :]))
        Mx = ((xx[None, :] >= x0[:, None]) & (xx[None, :] < x1[:, None]))
        Eoh = (cls[:, None] == np.arange(C)[None, :]).astype(np.float32)

        aspect = w.astype(np.float32) / h.astype(np.float32)
        ori = ori_wh[b, :, 0].astype(np.float32) / ori_wh[b, :, 1].astype(np.float32)
        bad = ~((aspect > 0.5 * ori) & (aspect < 2.0 * ori))
        badw = np.where(bad, 0.5, 1.0).astype(np.float32)
        valid = reg_mask[b] * (w * h > 0).astype(np.float32)

        # unique positive pixels (duplicated centers collapse in hm)
        flat = cls * (H * W) + cy * W + cx
        _, uidx = np.unique(flat, return_index=True)
        nu = len(uidx)
        cls_u, cy_u, cx_u = cls[uidx], cy[uidx], cx[uidx]
        inY = (cy_u[None, :] >= y0[:, None]) & (cy_u[None, :] < y1[:, None])
        inX = (cx_u[None, :] >= x0[:, None]) & (cx_u[None, :] < x1[:, None])
        sameC = cls[:, None] == cls_u[None, :]
        Mkj = (sameC & inY & inX).astype(np.float32)  # [k, j<nu]
        npos = Mkj.sum(1)
        MT = np.zeros((K, K), np.float32)
        MT[:nu, :] = Mkj.T
        # row in the y-major [(b y c), x] flattening of phm
        rpos_v = np.zeros(K, np.int32)
        rpos_v[:nu] = ((b % NB) * H + cy_u) * C + cls_u
        cxsel_v = np.zeros((K, W), np.float32)
        cx_pad = np.zeros(K, np.int64)
        cx_pad[:nu] = cx_u
        cxsel_v[np.arange(K), cx_pad] = 1.0

        r = np.where(npos > 0, 1.0 / np.maximum(npos, 1.0), 1.0)
        s = (-(r * badw * valid)).astype(np.float32)

        rr = ind[b] // W
        cind = ind[b] % W
        rind_v = ((b % NB) * H + rr).astype(np.int32)
        csind_v = np.zeros((K, W), np.float32)
        csind_v[np.arange(K), cind] = 1.0

        m = reg_mask[b]
        M2 = np.stack([m, m], 1).astype(np.float32)
        TMW = (wh_t[b] * m[:, None]).astype(np.float32)
        TMR = (reg_t[b] * m[:, None]).astype(np.float32)
        nobj = float(m.sum())
        c1 = (1.0 / max(nobj, 1.0)) if nobj > 0 else 1.0
        invden = 1.0 / (2.0 * nobj + 1e-4)

        fpk_v = np.zeros((K, F_TOT), np.float32)
        fpk_v[:, F_MXR : F_MXR + 4 * W] = np.tile(Mx, (1, 4))
        fpk_v[:, F_MX2 : F_MX2 + W // 2] = 0.5 * (Mx[:, 0::2] + Mx[:, 1::2])
        fpk_v[:, F_CX : F_CX + W] = cxsel_v
        fpk_v[:, F_CS : F_CS + W] = csind_v
        fpk_v[:, F_M2 : F_M2 + 2] = M2
        fpk_v[:, F_TW : F_TW + 2] = TMW
        fpk_v[:, F_TR : F_TR + 2] = TMR
        fpk_v[:, F_SK] = s
        bpk_v = np.zeros((K, B_TOT), BF16)
        bpk_v[:, B_MY : B_MY + K] = MyT.astype(BF16)
        bpk_v[:, B_MN : B_MN + K] = (-MyT.astype(np.float32)).astype(BF16)
        bpk_v[:, B_EO : B_EO + C] = Eoh.astype(BF16)
        bpk_v[:, B_MT : B_MT + K] = MT.astype(BF16)
        ipk_v = np.stack([rpos_v, rind_v], 1).astype(np.int32)

        per_img.append(dict(fpk=fpk_v, bpk=bpk_v, ipk=ipk_v, c1=c1, invden=invden))

    in_maps = []
    for core in range(NCORES):
        bs = [core * NB + j for j in range(NB)]
        pi = [per_img[b] for b in bs]
        in_maps.append(
            {
                "phm": np.ascontiguousarray(pred_hm[bs].transpose(0, 2, 1, 3)),
                "hm": np.ascontiguousarray(hm[bs].transpose(0, 2, 1, 3)),
                "pwh": np.ascontiguousarray(pred_wh[bs].transpose(0, 2, 1, 3)),
                "prg": np.ascontiguousarray(pred_reg[bs].transpose(0, 2, 1, 3)),
                "fpk": np.stack([p["fpk"] for p in pi]),
                "bpk": np.stack([p["bpk"] for p in pi]),
                "ipk": np.stack([p["ipk"] for p in pi]),
            }
        )
    aux = dict(
        c1=np.array([p["c1"] for p in per_img]),
        invden=np.array([p["invden"] for p in per_img]),
    )
    return in_maps, aux


def combine_outputs(outs, aux):
    """outs: list of 8 per-core 'out' arrays [4, NB]."""
    q = np.concatenate([o.T for o in outs], 0).astype(np.float64)  # [B, 4]
    q_hm, q_wh, q_rg = q[:, 0], q[:, 1], q[:, 2]
    wh_i = q_wh * aux["invden"]
    off_i = q_rg * aux["invden"]
    final_loss = np.mean(HM_W * q_hm + WH_W * wh_i + OFF_W * off_i)
    final_hm = np.mean(q_hm * aux["c1"])
    final_wh = np.mean(wh_i)
    final_off = np.mean(off_i)
    return (
        np.float32(final_loss),
        np.float32(final_hm),
        np.float32(final_wh),
        np.float32(final_off),
    )


def kernel(**inputs):
    from concourse.bass_utils import run_bass_kernel_spmd

    nc = build_module()
    in_maps, aux = prep_in_maps(inputs)
    res = run_bass_kernel_spmd(nc, in_maps, core_ids=list(range(NCORES)))
    outs = [r["out"] for r in res.results]
    return combine_outputs(outs, aux)

